# revision 6
# baseline (speedup 1.0000x reference)
"""Trainium2 Bass kernel for nn_EnhancedWaveletTransform2D.

Math (exact algebraic reductions of the reference):
  - wavedec2/waverec2 round trip == identity  ->  x_wave = x
  - conv(x*a) = a*conv(x) (depthwise), and InstanceNorm(affine=False) makes
    both the conv bias refine_b and any per-channel scale fold into the
    final affine:
        u   = depthwise_conv3x3(x)            (no bias, no attention scale)
        S_c = a_c / sqrt(a_c^2 * var(u_c) + eps)
        T_c = -mean(u_c) * S_c
        out = leaky_relu(u * S + T, 0.01)
    where a = sigmoid(W2 @ leaky_relu(W1 @ mean_spatial(x), 0.01)).

Sharding: pure data parallel, one sample (B=8) per NeuronCore (8 cores).

Per-core layout: channels (256 = 2 blocks of 128) on SBUF partitions,
pixels on the free dim. x streamed in 4 windows of 32 image rows (+1 halo
row each side, +1 zero pad column each side). Engines:
  - PE:  7 of 9 conv taps as float32r diagonal matmuls accumulating in PSUM
  - DVE: tap 8 (scalar_tensor_tensor in PSUM), tap 9 fused with PSUM->SBUF
         evacuation (+ accum_out = sum(u) for free)
  - ACT: Square pass (accum_out = sum(u^2)), global-avg-pool pass over x
         (Copy + accum_out), final fused normalize+leaky via Lrelu with
         per-partition scale/bias
"""
import os
import numpy as np

import concourse.tile as tile
from concourse import bacc, mybir
from concourse.bass_utils import run_bass_kernel_spmd

F32 = mybir.dt.float32
F32R = mybir.dt.float32r
BF16 = mybir.dt.bfloat16
AF = mybir.ActivationFunctionType
OP = mybir.AluOpType

C = 256
H = W = 128
HW = H * W
NBLK = 2          # channel blocks of 128
P = 128           # partitions
WIN_ROWS = 32     # output rows per streamed window
NWIN = H // WIN_ROWS
GRP_ROWS = 8      # output rows per psum group (1024 px = 2 psum banks)
NGRP_WIN = WIN_ROWS // GRP_ROWS
NGRP = H // GRP_ROWS          # 16 groups per block
SEG_ROWS = 4                  # rows per matmul (512 free dim = 1 bank)
NSEG = GRP_ROWS // SEG_ROWS   # 2 segs per group
EPS = 1e-5
SLOPE = 0.01
WPAD = W + 2                  # 130 padded columns
# tap order: (di, dj) row-major; last two go to DVE, first seven to PE
TAPS = [(di, dj) for di in (-1, 0, 1) for dj in (-1, 0, 1)]
PE_TAPS = TAPS[:7]
DVE_TAPS = TAPS[7:]


def _iteration(nc, pools, consts):
    """Trace one full sample-pipeline iteration."""
    xwin_pool, u_pool, sq_pool, p_pool, small, psum_pool, psum_misc = pools
    diag_sb, wcol_sb, w1t_sb, w2t_sb, eps_sb, x_d, y_d = consts

    pvec_sb = small.tile([P, NBLK], F32, tag="pvec", name="pvec")
    su_cols = [small.tile([P, NGRP], F32, tag=f"su{b}", name=f"su{b}") for b in range(NBLK)]
    ssq_cols = [small.tile([P, NGRP], F32, tag=f"ssq{b}", name=f"ssq{b}") for b in range(NBLK)]
    p_cols = [small.tile([P, NWIN * 4], F32, tag=f"pc{b}", name=f"pc{b}") for b in range(NBLK)]
    tch_v = small.tile([P, 2], F32, tag="tchv", name="tchv")
    tch_a = small.tile([P, 2], F32, tag="tcha", name="tcha")
    h_sb = small.tile([64, 1], F32, tag="hsb", name="hsb")
    a_sb = small.tile([P, NBLK], F32, tag="asb", name="asb")
    S_sb = small.tile([P, NBLK], F32, tag="Ssb", name="Ssb")
    T_sb = small.tile([P, NBLK], F32, tag="Tsb", name="Tsb")
    st_tmp = small.tile([P, 6], F32, tag="sttmp", name="sttmp")

    # absorb the wcol DMA waits on DVE (stt has one sync-wait slot)
    nc.vector.tensor_copy(out=tch_v[:, 1:2], in_=wcol_sb[:, 0:1])

    u_chunks = [[None] * NGRP for _ in range(NBLK)]

    # ---------------- conv + stats streaming ----------------
    for b in range(NBLK):
        for w in range(NWIN):
            r0 = w * WIN_ROWS
            xw = xwin_pool.tile([P, WIN_ROWS + 2, WPAD], F32R, tag="xw", name="xw")
            # zero the pad columns (and halo rows at image edges)
            nc.gpsimd.memset(xw[:, :, 0:1].bitcast(F32), 0.0)
            nc.gpsimd.memset(xw[:, :, WPAD - 1 : WPAD].bitcast(F32), 0.0)
            if w == 0:
                nc.gpsimd.memset(xw[:, 0:1, :].bitcast(F32), 0.0)
            if w == NWIN - 1:
                nc.gpsimd.memset(xw[:, WIN_ROWS + 1 : WIN_ROWS + 2, :].bitcast(F32), 0.0)
            src_lo = max(0, r0 - 1)
            src_hi = min(H, r0 + WIN_ROWS + 1)
            l0 = 1 if w == 0 else 0
            nc.gpsimd.dma_start(
                out=xw[:, l0 : l0 + (src_hi - src_lo), 1 : W + 1],
                in_=x_d[b, :, src_lo:src_hi, :],
            )
            # PE touch: dummy bf16 matmul absorbs xwin+diag DMA waits
            trash = psum_misc.tile([2, 2], F32, tag="m", name="trash")
            nc.tensor.matmul(
                out=trash,
                lhsT=diag_sb[b][:, 0, 0:1].bitcast(BF16),
                rhs=xw[:, 0:1, 0:1].bitcast(BF16),
                start=True,
                stop=True,
            )
            # DVE + ACT touches for the same reason
            nc.vector.tensor_copy(out=tch_v[:, 0:1], in_=xw[:, 0:1, 0:1].bitcast(F32))
            nc.scalar.copy(out=tch_a[:, 0:1], in_=xw[:, 0:1, 0:1].bitcast(F32))

            for gl in range(NGRP_WIN):
                gi = w * NGRP_WIN + gl
                ps = psum_pool.tile([P, GRP_ROWS * W], F32, tag="convps", name="convps")
                ps3 = ps.rearrange("p (r c) -> p r c", r=GRP_ROWS)
                # 7 taps on PE as f32r diagonal matmuls
                for ti, (di, dj) in enumerate(PE_TAPS):
                    for s in range(NSEG):
                        lrow = gl * GRP_ROWS + s * SEG_ROWS + 1 + di
                        rhs = xw[:, lrow : lrow + SEG_ROWS, 1 + dj : 1 + dj + W]
                        nc.tensor.matmul(
                            out=ps[:, s * SEG_ROWS * W : (s + 1) * SEG_ROWS * W],
                            lhsT=diag_sb[b][:, ti, :],
                            rhs=rhs,
                            start=(ti == 0),
                            stop=(ti == len(PE_TAPS) - 1),
                        )
                # tap 8 on DVE, accumulated in psum
                di, dj = DVE_TAPS[0]
                lrow = gl * GRP_ROWS + 1 + di
                nc.vector.scalar_tensor_tensor(
                    out=ps3,
                    in0=xw[:, lrow : lrow + GRP_ROWS, 1 + dj : 1 + dj + W].bitcast(F32),
                    scalar=wcol_sb[:, b * 9 + 7 : b * 9 + 8],
                    in1=ps3,
                    op0=OP.mult,
                    op1=OP.add,
                )
                # tap 9 on DVE, fused with evacuation to SBUF + sum(u)
                uc = u_pool.tile([P, GRP_ROWS * W], F32, tag="uc", name="uc")
                u_chunks[b][gi] = uc
                di, dj = DVE_TAPS[1]
                lrow = gl * GRP_ROWS + 1 + di
                nc.vector.scalar_tensor_tensor(
                    out=uc.rearrange("p (r c) -> p r c", r=GRP_ROWS),
                    in0=xw[:, lrow : lrow + GRP_ROWS, 1 + dj : 1 + dj + W].bitcast(F32),
                    scalar=wcol_sb[:, b * 9 + 8 : b * 9 + 9],
                    in1=ps3,
                    op0=OP.mult,
                    op1=OP.add,
                    accum_out=su_cols[b][:, gi : gi + 1],
                )
                # sum(u^2) on ACT: Square with accum_out
                sq = sq_pool.tile([P, GRP_ROWS * W], F32, tag="sq", name="sq")
                nc.scalar.activation(
                    out=sq,
                    in_=uc,
                    func=AF.Square,
                    accum_out=ssq_cols[b][:, gi : gi + 1],
                )
            # global-avg-pool partial sums over this window's owned rows
            # (quarters of 8 rows; pad columns are zero)
            for q in range(4):
                pd = p_pool.tile([P, 8 * WPAD], F32, tag="pd", name="pd")
                nc.scalar.activation(
                    out=pd.rearrange("p (r c) -> p r c", r=8),
                    in_=xw[:, 1 + 8 * q : 9 + 8 * q, :].bitcast(F32),
                    func=AF.Copy,
                    accum_out=p_cols[b][:, 4 * w + q : 4 * w + q + 1],
                )

    # ---------------- attention MLP ----------------
    for b in range(NBLK):
        nc.vector.reduce_sum(
            out=pvec_sb[:, b : b + 1], in_=p_cols[b], axis=mybir.AxisListType.X
        )
    # PE touch for w1t/pvec, then h = W1 @ p (fp32 matmuls, tiny)
    trash2 = psum_misc.tile([2, 2], F32, tag="m", name="trash2")
    nc.tensor.matmul(
        out=trash2,
        lhsT=w1t_sb[:, 0:1].bitcast(BF16),
        rhs=pvec_sb[:, 0:1].bitcast(BF16),
        start=True,
        stop=True,
    )
    h_ps = psum_misc.tile([64, 1], F32, tag="m", name="h_ps")
    for b in range(NBLK):
        nc.tensor.matmul(
            out=h_ps,
            lhsT=w1t_sb[:, b * 64 : (b + 1) * 64],
            rhs=pvec_sb[:, b : b + 1],
            start=(b == 0),
            stop=(b == NBLK - 1),
        )
    # h = leaky(h_ps / HW): fold the mean divide into the scale
    nc.scalar.activation(
        out=h_sb, in_=h_ps, func=AF.Lrelu, scale=1.0 / HW, alpha=SLOPE
    )
    # PE touch for w2t/h, then a = sigmoid(W2 @ h)
    trash3 = psum_misc.tile([2, 2], F32, tag="m", name="trash3")
    nc.tensor.matmul(
        out=trash3,
        lhsT=w2t_sb[:, 0:1].bitcast(BF16),
        rhs=h_sb[:, 0:1].bitcast(BF16),
        start=True,
        stop=True,
    )
    for b in range(NBLK):
        a_ps = psum_misc.tile([P, 1], F32, tag="m", name="a_ps")
        nc.tensor.matmul(
            out=a_ps,
            lhsT=w2t_sb[:, b * P : (b + 1) * P],
            rhs=h_sb,
            start=True,
            stop=True,
        )
        nc.scalar.activation(out=a_sb[:, b : b + 1], in_=a_ps, func=AF.Sigmoid)

    # ---------------- per-block affine S, T ----------------
    for b in range(NBLK):
        mean = st_tmp[:, 0:1]
        sumsq = st_tmp[:, 1:2]
        var = st_tmp[:, 2:3]
        asq = st_tmp[:, 3:4]
        v = st_tmp[:, 4:5]
        negmean = st_tmp[:, 5:6]
        # mean = sum(u)/HW  (reduce the 16 group sums, then scale)
        nc.vector.reduce_sum(out=mean, in_=su_cols[b], axis=mybir.AxisListType.X)
        nc.vector.tensor_scalar_mul(out=mean, in0=mean, scalar1=1.0 / HW)
        nc.vector.reduce_sum(out=sumsq, in_=ssq_cols[b], axis=mybir.AxisListType.X)
        # var = sumsq/HW - mean^2
        nc.vector.tensor_mul(out=v, in0=mean, in1=mean)
        nc.vector.scalar_tensor_tensor(
            out=var, in0=sumsq, scalar=1.0 / HW, in1=v,
            op0=OP.mult, op1=OP.subtract,
        )
        # v = a^2 * var + eps
        nc.vector.tensor_mul(out=asq, in0=a_sb[:, b : b + 1], in1=a_sb[:, b : b + 1])
        nc.vector.scalar_tensor_tensor(
            out=v, in0=var, scalar=asq, in1=eps_sb,
            op0=OP.mult, op1=OP.add,
        )
        # S = a / sqrt(v), T = -mean * S
        nc.scalar.activation(out=v, in_=v, func=AF.Sqrt)
        nc.vector.reciprocal(out=v, in_=v)
        nc.vector.tensor_mul(out=S_sb[:, b : b + 1], in0=a_sb[:, b : b + 1], in1=v)
        nc.vector.tensor_scalar_mul(out=negmean, in0=mean, scalar1=-1.0)
        nc.vector.tensor_mul(out=T_sb[:, b : b + 1], in0=negmean, in1=S_sb[:, b : b + 1])

    # ---------------- final normalize + leaky + store ----------------
    for b in range(NBLK):
        for gi in range(NGRP):
            uc = u_chunks[b][gi]
            nc.scalar.activation(
                out=uc,
                in_=uc,
                func=AF.Lrelu,
                bias=T_sb[:, b : b + 1],
                scale=S_sb[:, b : b + 1],
                alpha=SLOPE,
            )
            nc.gpsimd.dma_start(
                out=y_d[b, :, gi * GRP_ROWS : (gi + 1) * GRP_ROWS, :],
                in_=uc.rearrange("p (r c) -> p r c", r=GRP_ROWS),
            )


def build_nc(repeat=1):
    nc = bacc.Bacc("TRN2", target_bir_lowering=False)
    x_d = nc.declare_dram_parameter("x", [NBLK, P, H, W], F32R, isOutput=False)
    diag_d = nc.declare_dram_parameter("diag", [NBLK, P, 9, P], F32R, isOutput=False)
    wcol_d = nc.declare_dram_parameter("wcol", [P, NBLK * 9], F32, isOutput=False)
    w1t_d = nc.declare_dram_parameter("w1t", [P, 128], F32, isOutput=False)
    w2t_d = nc.declare_dram_parameter("w2t", [64, 256], F32, isOutput=False)
    y_d = nc.declare_dram_parameter("y", [NBLK, P, H, W], F32, isOutput=True)

    with tile.TileContext(nc) as tc:
        with (
            tc.tile_pool(name="xwin", bufs=2) as xwin_pool,
            tc.tile_pool(name="uchunks", bufs=NBLK * NGRP) as u_pool,
            tc.tile_pool(name="sqdump", bufs=2) as sq_pool,
            tc.tile_pool(name="pdump", bufs=2) as p_pool,
            tc.tile_pool(name="small", bufs=1) as small,
            tc.tile_pool(name="psum", bufs=3, space="PSUM") as psum_pool,
            tc.tile_pool(name="psum_misc", bufs=2, space="PSUM") as psum_misc,
        ):
            diag_sb = [small.tile([P, 9, P], F32R, tag=f"diag{b}", name=f"diag{b}") for b in range(NBLK)]
            wcol_sb = small.tile([P, NBLK * 9], F32, tag="wcol", name="wcol")
            w1t_sb = small.tile([P, 128], F32, tag="w1t", name="w1t")
            w2t_sb = small.tile([64, 256], F32, tag="w2t", name="w2t")
            eps_sb = small.tile([P, 1], F32, tag="eps", name="eps")
            for b in range(NBLK):
                nc.gpsimd.dma_start(out=diag_sb[b], in_=diag_d[b])
            nc.gpsimd.dma_start(out=wcol_sb, in_=wcol_d[:])
            nc.gpsimd.dma_start(out=w1t_sb, in_=w1t_d[:])
            nc.gpsimd.dma_start(out=w2t_sb, in_=w2t_d[:])
            nc.vector.memset(eps_sb, EPS)

            pools = (xwin_pool, u_pool, sq_pool, p_pool, small, psum_pool, psum_misc)
            consts = (diag_sb, wcol_sb, w1t_sb, w2t_sb, eps_sb, x_d, y_d)
            for _ in range(repeat):
                _iteration(nc, pools, consts)
    nc.compile()
    return nc


_NC_CACHE = {}


def _get_nc(repeat=1):
    if repeat not in _NC_CACHE:
        _NC_CACHE[repeat] = build_nc(repeat)
    return _NC_CACHE[repeat]


def make_in_maps(x, attn_w1, attn_w2, refine_w):
    """Host-side prep of per-core input maps (weights are tiny)."""
    B = x.shape[0]
    wt = refine_w.reshape(C, 9)                      # [256, 9] tap columns
    diag = np.zeros((NBLK, P, 9, P), np.float32)
    idx = np.arange(P)
    for b in range(NBLK):
        for t in range(9):
            diag[b, idx, t, idx] = wt[b * P : (b + 1) * P, t]
    wcol = np.empty((P, NBLK * 9), np.float32)
    for b in range(NBLK):
        wcol[:, b * 9 : (b + 1) * 9] = wt[b * P : (b + 1) * P, :]
    # w1t[k, b*64+m] = attn_w1[m, b*128+k]
    w1t = np.ascontiguousarray(
        attn_w1.T.reshape(NBLK, P, 64).transpose(1, 0, 2).reshape(P, 128)
    )
    w2t = np.ascontiguousarray(attn_w2.T)            # [64, 256]
    shared = {"diag": diag, "wcol": wcol, "w1t": w1t, "w2t": w2t}
    return [{"x": x[i].reshape(NBLK, P, H, W), **shared} for i in range(B)]


def run_nc(nc, in_maps):
    return run_bass_kernel_spmd(nc, in_maps, core_ids=list(range(len(in_maps))))


def kernel(x, attn_w1, attn_w2, refine_w, refine_b):
    x = np.asarray(x, dtype=np.float32)
    attn_w1 = np.asarray(attn_w1, dtype=np.float32)
    attn_w2 = np.asarray(attn_w2, dtype=np.float32)
    refine_w = np.asarray(refine_w, dtype=np.float32)
    B = x.shape[0]

    in_maps = make_in_maps(x, attn_w1, attn_w2, refine_w)
    nc = _get_nc(int(os.environ.get("KREPEAT", "1")))
    res = run_nc(nc, in_maps)
    out = np.stack([res.results[i]["y"].reshape(C, H, W) for i in range(B)])
    return out.astype(np.float32)
